# revision 1
# baseline (speedup 1.0000x reference)
"""BibdLinear Trainium2 kernel: out = input @ (weight * mask).T

Shapes (hardcoded): input [8192, 4096] f32, weight [4096, 4096] f32,
mask [4096, 4096] f32 -> out [8192, 4096] f32.

Sharding (column-parallel x batch-parallel, 8 cores):
  2 batch shards x 4 output-feature shards. Core c handles batch rows
  [(c//4)*4096, +4096) and output features [(c%4)*1024, +1024). Each core
  masks its weight slice on-device (DVE) and runs the GEMM on the tensor
  engine; host concatenates the 8 output slices.

Per-core device program (Bass/Tile):
  - inputs transposed on host to contraction-major: xT [4096, 4096],
    wT [4096, 1024] (both fed as float32r: full-rate fp32 matmul path,
    ~1.5e-4 rms vs fp32), mT [4096, 1024] bf16 (0/1 values — lossless).
  - mask multiply (DVE) produces 32 resident masked-weight k-strips
    [128, 1024]; production is interleaved with the first batch block's
    k-loop so the PE starts immediately.
  - GEMM: for each 512-row batch block, accumulate over 32 k-tiles into
    8 PSUM banks (4 batch subtiles x 2 feature chunks); lhsT = x k-tile
    [128,128] (stationary), rhs = masked-weight chunk [128,512] (moving).
  - evictions PSUM->SBUF on DVE, stores on the ACT DMA queue, x loads on
    the SP queue (separate queues so store bursts never starve x loads).
"""

import numpy as np
import ml_dtypes

import concourse.mybir as mybir
import concourse.tile as tile
from concourse import bacc
from concourse.bass_utils import run_bass_kernel_spmd

BATCH, IN_F, OUT_F = 8192, 4096, 4096
B_S, O_S = 2, 4                      # batch shards x out-feature shards
B, OF = BATCH // B_S, OUT_F // O_S   # 4096, 1024 per core
N_CORES = 8

F32 = mybir.dt.float32
F32R = mybir.dt.float32r
BF16 = mybir.dt.bfloat16

_NC_CACHE = {}


def _build_nc(NB=512, x_bufs=6, out_bufs=4, mask_bufs=3):
    K = IN_F
    KO = K // 128          # 32 contraction tiles
    B_SUB = NB // 128      # 4 batch subtiles per block
    OC = OF // 512         # 2 feature chunks
    NBLK = B // NB         # 8 batch blocks
    psum_bufs = 8 // (B_SUB * OC)

    nc = bacc.Bacc(None, target_bir_lowering=False)

    xT = nc.dram_tensor("xT", [K, B], F32R, kind="ExternalInput")
    wT = nc.dram_tensor("wT", [K, OF], F32R, kind="ExternalInput")
    mT = nc.dram_tensor("mT", [K, OF], BF16, kind="ExternalInput")
    out = nc.dram_tensor("out", [B, OF], F32, kind="ExternalOutput")

    xT3 = xT.rearrange("(ko p) b -> ko p b", p=128)
    wT3 = wT.rearrange("(ko p) o -> ko p o", p=128)
    mT3 = mT.rearrange("(ko p) o -> ko p o", p=128)

    with tile.TileContext(nc) as tc:
        with (
            tc.tile_pool(name="wpool", bufs=1) as wpool,
            tc.tile_pool(name="mpool", bufs=mask_bufs) as mpool,
            tc.tile_pool(name="xpool", bufs=x_bufs) as xpool,
            tc.tile_pool(name="opool", bufs=out_bufs) as opool,
            tc.tile_pool(name="psum", bufs=1, space="PSUM") as psum_pool,
        ):
            mw = [None] * KO

            def make_mw(k):
                mwt = wpool.tile([128, OF], F32R, tag=f"mw{k}", name=f"mw{k}")
                mtmp = mpool.tile([128, OF], BF16, tag="mtmp", name=f"mtmp{k}")
                nc.scalar.dma_start(mtmp, mT3[k])
                nc.scalar.dma_start(mwt, wT3[k])
                nc.vector.tensor_mul(mwt, mwt, mtmp)  # in-place mask
                mw[k] = mwt

            for bb in range(NBLK):
                psums = [
                    psum_pool.tile([128, 512], F32, tag=f"ps{i}",
                                   name=f"ps{i}_{bb}", bufs=psum_bufs)
                    for i in range(B_SUB * OC)
                ]
                for k in range(KO):
                    if bb == 0:
                        make_mw(k)  # interleave mask production with block 0
                    xt = xpool.tile([128, NB], F32R, tag="xt", name=f"xt{bb}_{k}")
                    nc.sync.dma_start(xt, xT3[k, :, bb * NB:(bb + 1) * NB])
                    for bs in range(B_SUB):
                        lhsT = xt[:, bs * 128:(bs + 1) * 128]
                        for oc in range(OC):
                            nc.tensor.matmul(
                                psums[bs * OC + oc], lhsT,
                                mw[k][:, oc * 512:(oc + 1) * 512],
                                start=(k == 0), stop=(k == KO - 1),
                            )
                for bs in range(B_SUB):
                    ot = opool.tile([128, OF], F32, tag="ot", name=f"ot{bb}_{bs}")
                    for oc in range(OC):
                        nc.vector.tensor_copy(
                            ot[:, oc * 512:(oc + 1) * 512], psums[bs * OC + oc]
                        )
                    nc.scalar.dma_start(
                        out[bb * NB + bs * 128: bb * NB + (bs + 1) * 128, :], ot
                    )

    nc.compile()
    return nc


def _get_nc():
    if "nc" not in _NC_CACHE:
        _NC_CACHE["nc"] = _build_nc()
    return _NC_CACHE["nc"]


def shard_inputs(input, weight, mask):
    """Host-side sharding/layout: per-core contraction-major slices."""
    x = np.ascontiguousarray(np.asarray(input, dtype=np.float32))
    w = np.ascontiguousarray(np.asarray(weight, dtype=np.float32))
    m = np.asarray(mask, dtype=np.float32)
    in_maps = []
    for c in range(N_CORES):
        b0 = (c // O_S) * B
        o0 = (c % O_S) * OF
        in_maps.append({
            "xT": np.ascontiguousarray(x[b0:b0 + B, :].T),
            "wT": np.ascontiguousarray(w[o0:o0 + OF, :].T),
            "mT": np.ascontiguousarray(m[o0:o0 + OF, :].T).astype(
                ml_dtypes.bfloat16),
        })
    return in_maps


def gather_output(results):
    outp = np.empty((BATCH, OUT_F), np.float32)
    for c in range(N_CORES):
        b0 = (c // O_S) * B
        o0 = (c % O_S) * OF
        outp[b0:b0 + B, o0:o0 + OF] = results[c]["out"]
    return outp


def kernel(input, weight, mask):
    in_maps = shard_inputs(input, weight, mask)
    res = run_bass_kernel_spmd(_get_nc(), in_maps, core_ids=list(range(N_CORES)))
    return gather_output(res.results)
